# revision 6
# baseline (speedup 1.0000x reference)
"""GCN (2-layer, PyG GCNConv semantics) on 8 Trainium2 NeuronCores.

Strategy (sharding_hint: shard nodes across cores, partition edges by dst):
  - Nodes sharded contiguously: core c owns dst rows [c*NP, (c+1)*NP).
  - Layer matmuls computed on the owning core (fp16 operands, fp32 PSUM).
  - Hidden tables (h1 = x@W1, h2 = relu(z1)@W2) are AllGathered so every
    core can gather messages for its own edges locally.
  - Aggregation out[dst] += norm_e * h[src_e] is done per 128-dst window:
    dma_gather pulls h[src] rows for the window's edges into SBUF (128
    edges per chunk), a selection matrix G[e, d] = (dst_rel[e]==d)*norm_e
    is built with one fused DVE tensor_scalar, and TensorE accumulates
    G.T @ msg into the window's PSUM tile.  Bias is the K=1 matmul
    ones.T @ b that also initializes the accumulation group.
  - All cores run one identical program: every (window, src-half) edge
    group is padded to CH chunks of 128 tokens (pad tokens have norm 0 so
    they contribute nothing).
"""

import math

import numpy as np

M = 8  # cores
P = 128  # partitions


def _prep(x, W1, b1, W2, b2, edge_index):
    """Host-side sharding/layout (index manipulation + dtype casts only).

    Returns (in_maps, meta) where meta drives the static program structure.
    """
    N, IN = x.shape
    HID = W1.shape[1]
    OUT = W2.shape[1]
    OUTP = P  # h2 rows padded to 128 cols so gather elems are 256B
    assert N % M == 0
    NP = N // M
    NPAD = math.ceil(NP / P) * P
    NT = NPAD // P  # node tiles == dst windows per core
    VROWS = M * NPAD
    HALF = VROWS // 2
    assert HALF <= 32768, "gather idx must fit int16"

    src = np.concatenate([edge_index[0], np.arange(N, dtype=np.int64)])
    dst = np.concatenate([edge_index[1], np.arange(N, dtype=np.int64)])
    deg = np.bincount(dst, minlength=N).astype(np.float32)
    dis = 1.0 / np.sqrt(np.maximum(deg, 1.0))
    norm = (dis[src] * dis[dst]).astype(np.float32)

    owner = dst // NP
    ldst = (dst - owner * NP).astype(np.int64)
    win = ldst // P
    g = (src // NP) * NPAD + (src % NP)  # padded global row of src
    half = (g // HALF).astype(np.int64)
    lidx = (g - half * HALF).astype(np.int64)

    # group id per edge: (core, window, half) -> sort edges into groups
    gid = (owner * NT + win) * 2 + half
    order = np.argsort(gid, kind="stable")
    gid_s = gid[order]
    lidx_s = lidx[order]
    drel_s = (ldst[order] % P).astype(np.float32)
    norm_s = norm[order]

    ngroups = M * NT * 2
    counts = np.bincount(gid_s, minlength=ngroups)
    CH = max(1, int(math.ceil(counts.max() / P)))
    GTOK = CH * P  # tokens per (window, half) group
    TOK = NT * 2 * GTOK  # tokens per core
    NCHUNK = NT * 2 * CH

    # batches of windows (gathers batched over BW windows at a time)
    BW = 4
    batches = [list(range(s, min(s + BW, NT))) for s in range(0, NT, BW)]

    # token slot base for each (window, half) group, honoring the kernel's
    # iteration order: batch -> half -> window -> chunk
    slot_base = np.zeros((NT, 2), dtype=np.int64)
    tb = 0
    for bwins in batches:
        bw = len(bwins)
        for h in (0, 1):
            for i, w in enumerate(bwins):
                slot_base[w, h] = tb + i * GTOK
            tb += bw * GTOK
    assert tb == TOK

    group_starts = np.zeros(ngroups + 1, dtype=np.int64)
    np.cumsum(counts, out=group_starts[1:])

    in_maps = []
    f16 = np.float16
    w1f = np.ascontiguousarray(W1, dtype=f16)
    b1f = np.asarray(b1, dtype=f16).reshape(1, HID)
    w2f = np.zeros((HID, OUTP), dtype=f16)
    w2f[:, :OUT] = W2.astype(f16)
    b2f = np.zeros((1, OUTP), dtype=f16)
    b2f[0, :OUT] = np.asarray(b2, dtype=f16)
    iota_np = np.ascontiguousarray(
        np.broadcast_to(np.arange(P, dtype=f16), (P, P))
    )

    for c in range(M):
        xt = np.zeros((IN, NPAD), dtype=f16)
        xt[:, :NP] = x[c * NP : (c + 1) * NP].T
        idx16 = np.zeros(TOK, dtype=np.int16)
        drel = np.full(TOK, 255.0, dtype=np.float32)
        nrm = np.zeros(TOK, dtype=np.float32)
        for w in range(NT):
            for h in (0, 1):
                gi = (c * NT + w) * 2 + h
                s0, s1 = group_starts[gi], group_starts[gi + 1]
                k = s1 - s0
                base = slot_base[w, h]
                idx16[base : base + k] = lidx_s[s0:s1]
                drel[base : base + k] = drel_s[s0:s1]
                nrm[base : base + k] = norm_s[s0:s1]
        idx_w = np.tile(idx16.reshape(-1, 16).T, (8, 1))  # [128, TOK//16]
        grel = np.ascontiguousarray(drel.reshape(-1, P).T)  # [128, NCHUNK]
        gnrm = np.ascontiguousarray(nrm.reshape(-1, P).T)
        in_maps.append(
            {
                "xt": xt,
                "w1": w1f,
                "b1": b1f,
                "w2": w2f,
                "b2": b2f,
                "iota": np.array(iota_np),
                "idx": np.ascontiguousarray(idx_w),
                "grel": grel,
                "gnrm": gnrm,
            }
        )

    meta = dict(
        N=N, IN=IN, HID=HID, OUT=OUT, OUTP=OUTP, NP=NP, NPAD=NPAD, NT=NT,
        VROWS=VROWS, HALF=HALF, CH=CH, TOK=TOK, NCHUNK=NCHUNK,
        batches=batches,
    )
    return in_maps, meta


def _build(meta):
    import os

    import concourse.bass as bass
    import concourse.mybir as mybir
    import concourse.tile as tile
    from concourse import bacc
    from concourse.bass import ts
    from concourse.masks import make_identity

    stages = os.environ.get("GCN_STAGES", "12345")

    IN, HID, OUT, OUTP = meta["IN"], meta["HID"], meta["OUT"], meta["OUTP"]
    NPAD, NT, VROWS, HALF = meta["NPAD"], meta["NT"], meta["VROWS"], meta["HALF"]
    CH, TOK, NCHUNK = meta["CH"], meta["TOK"], meta["NCHUNK"]
    batches = meta["batches"]
    KT = IN // P
    HC = HID // P
    f16 = mybir.dt.float16
    f32 = mybir.dt.float32

    NQ = 4  # SWDGE queues for gather descriptor rings
    GN = 1024  # max gather tokens per instruction (descriptor ring is 1024)
    nc = bacc.Bacc(
        "TRN2",
        target_bir_lowering=False,
        debug=False,
        num_devices=M,
        num_swdge_queues=NQ,
    )

    xt_d = nc.dram_tensor("xt", [IN, NPAD], f16, kind="ExternalInput")
    w1_d = nc.dram_tensor("w1", [IN, HID], f16, kind="ExternalInput")
    b1_d = nc.dram_tensor("b1", [1, HID], f16, kind="ExternalInput")
    w2_d = nc.dram_tensor("w2", [HID, OUTP], f16, kind="ExternalInput")
    b2_d = nc.dram_tensor("b2", [1, OUTP], f16, kind="ExternalInput")
    iota_d = nc.dram_tensor("iota", [P, P], f16, kind="ExternalInput")
    idx_d = nc.dram_tensor("idx", [P, TOK // 16], mybir.dt.int16, kind="ExternalInput")
    grel_d = nc.dram_tensor("grel", [P, NCHUNK], f32, kind="ExternalInput")
    gnrm_d = nc.dram_tensor("gnrm", [P, NCHUNK], f32, kind="ExternalInput")
    out_d = nc.dram_tensor("out", [NPAD, OUT], f32, kind="ExternalOutput")

    h1_loc = nc.dram_tensor("h1_loc", [NPAD, HID], f16)
    h1_gl = nc.dram_tensor("h1_gl", [VROWS, HID], f16, addr_space="Shared")
    h2_loc = nc.dram_tensor("h2_loc", [NPAD, OUTP], f16)
    h2_gl = nc.dram_tensor("h2_gl", [VROWS, OUTP], f16, addr_space="Shared")

    rg = [list(range(M))]

    with tile.TileContext(nc) as tc:
        with (
            tc.tile_pool(name="const", bufs=1) as cp,
            tc.tile_pool(name="work", bufs=3) as wp,
            tc.tile_pool(name="gpool", bufs=6) as gp,
            tc.tile_pool(name="idxp", bufs=2) as idxp,
            tc.tile_pool(name="psum", bufs=2, space="PSUM") as pp,
        ):
            # ---- constants ----
            w1t = cp.tile([P, KT, HID], f16)
            nc.sync.dma_start(
                out=w1t[:], in_=w1_d[:, :].rearrange("(k p) h -> p k h", p=P)
            )
            w2t = cp.tile([P, HC, OUTP], f16)
            nc.sync.dma_start(
                out=w2t[:], in_=w2_d[:, :].rearrange("(k p) o -> p k o", p=P)
            )
            iota_t = cp.tile([P, P], f16)
            nc.sync.dma_start(out=iota_t[:], in_=iota_d[:, :])
            ident = cp.tile([P, P], f16)
            make_identity(nc, ident[:])
            ones = cp.tile([1, P], f16)
            nc.gpsimd.memset(ones[:], 1.0)
            b1s = cp.tile([1, HID], f16)
            nc.sync.dma_start(out=b1s[:], in_=b1_d[:, :])
            b2s = cp.tile([1, OUTP], f16)
            nc.sync.dma_start(out=b2s[:], in_=b2_d[:, :])
            grelS = cp.tile([P, NCHUNK], f32)
            nc.sync.dma_start(out=grelS[:], in_=grel_d[:, :])
            gnrmS = cp.tile([P, NCHUNK], f32)
            nc.sync.dma_start(out=gnrmS[:], in_=gnrm_d[:, :])

            # ---- stage 1: h1 = x @ W1 (node-tile at a time) ----
            for nt in range(NT if "1" in stages else 0):
                xtt = wp.tile([P, KT, P], f16, tag="xtt")
                nc.sync.dma_start(
                    out=xtt[:],
                    in_=xt_d[:, ts(nt, P)].rearrange("(k p) n -> p k n", p=P),
                )
                ph = pp.tile([P, HID], f32, tag="acc256")
                for k in range(KT):
                    nc.tensor.matmul(
                        ph[:],
                        lhsT=xtt[:, k, :],
                        rhs=w1t[:, k, :],
                        start=(k == 0),
                        stop=(k == KT - 1),
                    )
                h1s = wp.tile([P, HID], f16, tag="h1s")
                nc.scalar.activation(h1s[:], ph[:], mybir.ActivationFunctionType.Copy)
                nc.sync.dma_start(out=h1_loc[ts(nt, P), :], in_=h1s[:])

            # ---- stage 2: AllGather h1 ----
            if "2" in stages:
                nc.gpsimd.collective_compute(
                    "AllGather",
                    mybir.AluOpType.bypass,
                    replica_groups=rg,
                    ins=[h1_loc.ap().opt()],
                    outs=[h1_gl.ap().opt()],
                )

            # ---- stage 3: layer-1 aggregation + fused layer-2 dense ----
            qctr = [0]

            def agg_batches(table, table_elem, msg_tag, msg_pool, consume_window):
                tok_base = 0
                col_base = 0
                for bwins in batches:
                    bw = len(bwins)
                    btok = bw * CH * P
                    idx_t = idxp.tile([P, 2 * btok // 16], mybir.dt.int16, tag="idx")
                    nc.sync.dma_start(
                        out=idx_t[:],
                        in_=idx_d[:, tok_base // 16 : (tok_base + 2 * btok) // 16],
                    )
                    msgs = []
                    for h in (0, 1):
                        mt = msg_pool.tile([P, bw * CH, table_elem], f16, tag=msg_tag)
                        lo = h * HALF
                        for off in range(0, btok, GN):
                            gn = min(GN, btok - off)
                            i0 = h * btok + off
                            nc.gpsimd.dma_gather(
                                out_ap=mt[:, off // P : (off + gn) // P, :],
                                in_ap=table[lo : lo + HALF, :],
                                idxs_ap=idx_t[:, i0 // 16 : (i0 + gn) // 16],
                                num_idxs=gn,
                                num_idxs_reg=gn,
                                elem_size=table_elem,
                                queue_num=qctr[0] % NQ,
                            )
                            qctr[0] += 1
                        msgs.append(mt)
                    for i, w in enumerate(bwins):
                        consume_window(w, i, bw, msgs, col_base)
                    tok_base += 2 * btok
                    col_base += 2 * bw * CH

            def build_G(col):
                G = gp.tile([P, P], f16, tag="G")
                nc.vector.tensor_scalar(
                    out=G[:],
                    in0=iota_t[:],
                    scalar1=grelS[:, col : col + 1],
                    scalar2=gnrmS[:, col : col + 1],
                    op0=mybir.AluOpType.is_equal,
                    op1=mybir.AluOpType.mult,
                )
                return G

            def stage3_window(w, i, bw, msgs, col_base):
                pz = pp.tile([P, HID], f32, tag="acc256")
                nc.tensor.matmul(
                    pz[:], lhsT=ones[:1, :], rhs=b1s[:1, :], start=True, stop=False
                )
                for h in (0, 1):
                    for c in range(CH):
                        col = col_base + (h * bw + i) * CH + c
                        G = build_G(col)
                        nc.tensor.matmul(
                            pz[:],
                            lhsT=G[:],
                            rhs=msgs[h][:, i * CH + c, :],
                            start=False,
                            stop=(h == 1 and c == CH - 1),
                        )
                z1r = wp.tile([P, HID], f16, tag="z1r")
                nc.scalar.activation(
                    z1r[:], pz[:], mybir.ActivationFunctionType.Relu
                )
                ph2 = pp.tile([P, OUTP], f32, tag="acc128b")
                for k in range(HC):
                    pt = pp.tile([P, P], f16, tag="acc128t")
                    nc.tensor.transpose(pt[:], z1r[:, ts(k, P)], ident[:])
                    zt = wp.tile([P, P], f16, tag="zt")
                    nc.vector.tensor_copy(zt[:], pt[:])
                    nc.tensor.matmul(
                        ph2[:],
                        lhsT=zt[:],
                        rhs=w2t[:, k, :],
                        start=(k == 0),
                        stop=(k == HC - 1),
                    )
                h2s = wp.tile([P, OUTP], f16, tag="h2s")
                nc.scalar.activation(h2s[:], ph2[:], mybir.ActivationFunctionType.Copy)
                nc.sync.dma_start(out=h2_loc[ts(w, P), :], in_=h2s[:])

            if "3" in stages:
                with tc.tile_pool(name="msg1", bufs=4) as mp1:
                    agg_batches(h1_gl, HID, "m1", mp1, stage3_window)

            # ---- stage 4: AllGather h2 ----
            if "4" in stages:
                nc.gpsimd.collective_compute(
                    "AllGather",
                    mybir.AluOpType.bypass,
                    replica_groups=rg,
                    ins=[h2_loc.ap().opt()],
                    outs=[h2_gl.ap().opt()],
                )

            # ---- stage 5: layer-2 aggregation ----
            def stage5_window(w, i, bw, msgs, col_base):
                po = pp.tile([P, OUTP], f32, tag="acc128b")
                nc.tensor.matmul(
                    po[:], lhsT=ones[:1, :], rhs=b2s[:1, :], start=True, stop=False
                )
                for h in (0, 1):
                    for c in range(CH):
                        col = col_base + (h * bw + i) * CH + c
                        G = build_G(col)
                        nc.tensor.matmul(
                            po[:],
                            lhsT=G[:],
                            rhs=msgs[h][:, i * CH + c, :],
                            start=False,
                            stop=(h == 1 and c == CH - 1),
                        )
                os_ = wp.tile([P, OUT], f32, tag="os")
                nc.vector.tensor_copy(os_[:], po[:, :OUT])
                nc.sync.dma_start(out=out_d[ts(w, P), :], in_=os_[:])

            if "5" in stages:
                with tc.tile_pool(name="msg2", bufs=4) as mp2:
                    agg_batches(h2_gl, OUTP, "m2", mp2, stage5_window)
            else:
                zf = wp.tile([P, OUT], f32, tag="os")
                nc.gpsimd.memset(zf[:], 0.0)
                nc.sync.dma_start(out=out_d[0:P, :], in_=zf[:])

    nc.compile()
    return nc


def kernel(x, W1, b1, W2, b2, edge_index, _run_opts=None):
    from concourse.bass_utils import run_bass_kernel_spmd

    x = np.asarray(x)
    edge_index = np.asarray(edge_index)
    in_maps, meta = _prep(
        x, np.asarray(W1), np.asarray(b1), np.asarray(W2), np.asarray(b2), edge_index
    )
    nc = _build(meta)
    opts = _run_opts or {}
    res = run_bass_kernel_spmd(nc, in_maps, core_ids=list(range(M)), **opts)
    N, NP, OUT = meta["N"], meta["NP"], meta["OUT"]
    out = np.concatenate(
        [res.results[c]["out"][:NP] for c in range(M)], axis=0
    ).astype(np.float32)
    if _run_opts is not None:
        _run_opts["_bass_results"] = res
    return out


# revision 7
# speedup vs baseline: 1.0284x; 1.0284x over previous
"""GCN (2-layer, PyG GCNConv semantics) on 8 Trainium2 NeuronCores.

Strategy (sharding_hint: shard nodes across cores, partition edges by dst):
  - Nodes sharded contiguously: core c owns dst rows [c*NP, (c+1)*NP).
  - Layer matmuls computed on the owning core (fp16 operands, fp32 PSUM).
  - Hidden tables (h1 = x@W1, h2 = relu(z1)@W2) are AllGathered so every
    core can gather messages for its own edges locally.
  - Aggregation out[dst] += norm_e * h[src_e] is done per 128-dst window:
    dma_gather pulls h[src] rows for the window's edges into SBUF (128
    edges per chunk), a selection matrix G[e, d] = (dst_rel[e]==d)*norm_e
    is built with one fused DVE tensor_scalar, and TensorE accumulates
    G.T @ msg into the window's PSUM tile.  Bias is the K=1 matmul
    ones.T @ b that also initializes the accumulation group.
  - All cores run one identical program: every (window, src-half) edge
    group is padded to CH chunks of 128 tokens (pad tokens have norm 0 so
    they contribute nothing).
"""

import math

import numpy as np

M = 8  # cores
P = 128  # partitions


def _prep(x, W1, b1, W2, b2, edge_index):
    """Host-side sharding/layout (index manipulation + dtype casts only).

    Returns (in_maps, meta) where meta drives the static program structure.
    """
    N, IN = x.shape
    HID = W1.shape[1]
    OUT = W2.shape[1]
    OUTP = P  # h2 rows padded to 128 cols so gather elems are 256B
    assert N % M == 0
    NP = N // M
    NPAD = math.ceil(NP / P) * P
    NT = NPAD // P  # node tiles == dst windows per core
    VROWS = M * NPAD
    HALF = VROWS // 2
    assert HALF <= 32768, "gather idx must fit int16"

    src = np.concatenate([edge_index[0], np.arange(N, dtype=np.int64)])
    dst = np.concatenate([edge_index[1], np.arange(N, dtype=np.int64)])
    deg = np.bincount(dst, minlength=N).astype(np.float32)
    dis = 1.0 / np.sqrt(np.maximum(deg, 1.0))

    owner = dst // NP
    ldst = (dst - owner * NP).astype(np.int64)
    win = ldst // P
    g = (src // NP) * NPAD + (src % NP)  # padded global row of src
    half = (g // HALF).astype(np.int64)
    lidx = (g - half * HALF).astype(np.int64)

    # group id per edge: (core, window, half) -> sort edges into groups
    gid = (owner * NT + win) * 2 + half
    order = np.argsort(gid, kind="stable")
    gid_s = gid[order]
    lidx_s = lidx[order]
    drel_s = (ldst[order] % P).astype(np.float16)

    ngroups = M * NT * 2
    counts = np.bincount(gid_s, minlength=ngroups)
    CH = max(1, int(math.ceil(counts.max() / P)))
    GTOK = CH * P  # tokens per (window, half) group
    TOK = NT * 2 * GTOK  # tokens per core
    NCHUNK = NT * 2 * CH

    # batches of windows (gathers batched over BW windows at a time)
    BW = 4
    batches = [list(range(s, min(s + BW, NT))) for s in range(0, NT, BW)]

    # token slot base for each (window, half) group, honoring the kernel's
    # iteration order: batch -> half -> window -> chunk
    slot_base = np.zeros((NT, 2), dtype=np.int64)
    tb = 0
    for bwins in batches:
        bw = len(bwins)
        for h in (0, 1):
            for i, w in enumerate(bwins):
                slot_base[w, h] = tb + i * GTOK
            tb += bw * GTOK
    assert tb == TOK

    group_starts = np.zeros(ngroups + 1, dtype=np.int64)
    np.cumsum(counts, out=group_starts[1:])

    in_maps = []
    f16 = np.float16
    w1f = np.ascontiguousarray(W1, dtype=f16)
    b1f = np.asarray(b1, dtype=f16).reshape(1, HID)
    w2f = np.zeros((HID, OUTP), dtype=f16)
    w2f[:, :OUT] = W2.astype(f16)
    b2f = np.zeros((1, OUTP), dtype=f16)
    b2f[0, :OUT] = np.asarray(b2, dtype=f16)
    iota_np = np.ascontiguousarray(
        np.broadcast_to(np.arange(P, dtype=f16), (P, P))
    )

    for c in range(M):
        xt = np.zeros((IN, NPAD), dtype=f16)
        xt[:, :NP] = x[c * NP : (c + 1) * NP].T
        idx16 = np.zeros(TOK, dtype=np.int16)
        drel = np.full(TOK, 255.0, dtype=np.float16)
        for w in range(NT):
            for h in (0, 1):
                gi = (c * NT + w) * 2 + h
                s0, s1 = group_starts[gi], group_starts[gi + 1]
                k = s1 - s0
                base = slot_base[w, h]
                idx16[base : base + k] = lidx_s[s0:s1]
                drel[base : base + k] = drel_s[s0:s1]
        idx_w = np.tile(idx16.reshape(-1, 16).T, (8, 1))  # [128, TOK//16]
        grel = np.ascontiguousarray(drel.reshape(-1, P).T)  # [128, NCHUNK]
        dloc = np.ones(NPAD, np.float32)
        dloc[:NP] = dis[c * NP : (c + 1) * NP]
        disS = np.ascontiguousarray(dloc.reshape(NT, P).T)  # [128, NT] f32
        invd = (1.0 / dloc).astype(f16).reshape(1, NPAD)
        in_maps.append(
            {
                "xt": xt,
                "w1": w1f,
                "b1": b1f,
                "w2": w2f,
                "b2": b2f,
                "iota": np.array(iota_np),
                "idx": np.ascontiguousarray(idx_w),
                "grel": grel,
                "disS": disS,
                "invd": invd,
            }
        )

    meta = dict(
        N=N, IN=IN, HID=HID, OUT=OUT, OUTP=OUTP, NP=NP, NPAD=NPAD, NT=NT,
        VROWS=VROWS, HALF=HALF, CH=CH, TOK=TOK, NCHUNK=NCHUNK,
        batches=batches,
    )
    return in_maps, meta


def _build(meta):
    import os

    import concourse.bass as bass
    import concourse.mybir as mybir
    import concourse.tile as tile
    from concourse import bacc
    from concourse.bass import ts
    from concourse.masks import make_identity

    stages = os.environ.get("GCN_STAGES", "12345")

    IN, HID, OUT, OUTP = meta["IN"], meta["HID"], meta["OUT"], meta["OUTP"]
    NPAD, NT, VROWS, HALF = meta["NPAD"], meta["NT"], meta["VROWS"], meta["HALF"]
    CH, TOK, NCHUNK = meta["CH"], meta["TOK"], meta["NCHUNK"]
    batches = meta["batches"]
    KT = IN // P
    HC = HID // P
    f16 = mybir.dt.float16
    f32 = mybir.dt.float32

    NQ = 4  # SWDGE queues for gather descriptor rings
    GN = 1024  # max gather tokens per instruction (descriptor ring is 1024)
    nc = bacc.Bacc(
        "TRN2",
        target_bir_lowering=False,
        debug=False,
        num_devices=M,
        num_swdge_queues=NQ,
    )

    xt_d = nc.dram_tensor("xt", [IN, NPAD], f16, kind="ExternalInput")
    w1_d = nc.dram_tensor("w1", [IN, HID], f16, kind="ExternalInput")
    b1_d = nc.dram_tensor("b1", [1, HID], f16, kind="ExternalInput")
    w2_d = nc.dram_tensor("w2", [HID, OUTP], f16, kind="ExternalInput")
    b2_d = nc.dram_tensor("b2", [1, OUTP], f16, kind="ExternalInput")
    iota_d = nc.dram_tensor("iota", [P, P], f16, kind="ExternalInput")
    idx_d = nc.dram_tensor("idx", [P, TOK // 16], mybir.dt.int16, kind="ExternalInput")
    grel_d = nc.dram_tensor("grel", [P, NCHUNK], f16, kind="ExternalInput")
    disS_d = nc.dram_tensor("disS", [P, NT], f32, kind="ExternalInput")
    invd_d = nc.dram_tensor("invd", [1, NPAD], f16, kind="ExternalInput")
    out_d = nc.dram_tensor("out", [NPAD, OUT], f32, kind="ExternalOutput")

    h1_loc = nc.dram_tensor("h1_loc", [NPAD, HID], f16)
    h1_gl = nc.dram_tensor("h1_gl", [VROWS, HID], f16, addr_space="Shared")
    h2_loc = nc.dram_tensor("h2_loc", [NPAD, OUTP], f16)
    h2_gl = nc.dram_tensor("h2_gl", [VROWS, OUTP], f16, addr_space="Shared")

    rg = [list(range(M))]

    with tile.TileContext(nc) as tc:
        with (
            tc.tile_pool(name="const", bufs=1) as cp,
            tc.tile_pool(name="work", bufs=3) as wp,
            tc.tile_pool(name="gpool", bufs=6) as gp,
            tc.tile_pool(name="idxp", bufs=2) as idxp,
            tc.tile_pool(name="psum", bufs=2, space="PSUM") as pp,
        ):
            # ---- constants ----
            w1t = cp.tile([P, KT, HID], f16)
            nc.sync.dma_start(
                out=w1t[:], in_=w1_d[:, :].rearrange("(k p) h -> p k h", p=P)
            )
            w2t = cp.tile([P, HC, OUTP], f16)
            nc.sync.dma_start(
                out=w2t[:], in_=w2_d[:, :].rearrange("(k p) o -> p k o", p=P)
            )
            iota_t = cp.tile([P, P], f16)
            nc.sync.dma_start(out=iota_t[:], in_=iota_d[:, :])
            ident = cp.tile([P, P], f16)
            make_identity(nc, ident[:])
            ones = cp.tile([1, P], f16)
            nc.gpsimd.memset(ones[:], 1.0)
            b1s = cp.tile([1, HID], f16)
            nc.sync.dma_start(out=b1s[:], in_=b1_d[:, :])
            b2s = cp.tile([1, OUTP], f16)
            nc.sync.dma_start(out=b2s[:], in_=b2_d[:, :])
            grelS = cp.tile([P, NCHUNK], f16)
            nc.sync.dma_start(out=grelS[:], in_=grel_d[:, :])
            disS = cp.tile([P, NT], f32)
            nc.sync.dma_start(out=disS[:], in_=disS_d[:, :])
            invd = cp.tile([1, NPAD], f16)
            nc.sync.dma_start(out=invd[:], in_=invd_d[:, :])

            # ---- stage 1: h1 = x @ W1 (node-tile at a time) ----
            for nt in range(NT if "1" in stages else 0):
                xtt = wp.tile([P, KT, P], f16, tag="xtt")
                nc.sync.dma_start(
                    out=xtt[:],
                    in_=xt_d[:, ts(nt, P)].rearrange("(k p) n -> p k n", p=P),
                )
                ph = pp.tile([P, HID], f32, tag="acc256")
                for k in range(KT):
                    nc.tensor.matmul(
                        ph[:],
                        lhsT=xtt[:, k, :],
                        rhs=w1t[:, k, :],
                        start=(k == 0),
                        stop=(k == KT - 1),
                    )
                h1s = wp.tile([P, HID], f16, tag="h1s")
                nc.scalar.activation(
                    h1s[:], ph[:], mybir.ActivationFunctionType.Copy,
                    scale=disS[:, nt : nt + 1],
                )
                nc.sync.dma_start(out=h1_loc[ts(nt, P), :], in_=h1s[:])

            # ---- stage 2: AllGather h1 ----
            if "2" in stages:
                nc.gpsimd.collective_compute(
                    "AllGather",
                    mybir.AluOpType.bypass,
                    replica_groups=rg,
                    ins=[h1_loc.ap().opt()],
                    outs=[h1_gl.ap().opt()],
                )

            # ---- stage 3: layer-1 aggregation + fused layer-2 dense ----
            qctr = [0]

            def agg_batches(table, table_elem, msg_tag, msg_pool, consume_window):
                tok_base = 0
                col_base = 0
                for bwins in batches:
                    bw = len(bwins)
                    btok = bw * CH * P
                    idx_t = idxp.tile([P, 2 * btok // 16], mybir.dt.int16, tag="idx")
                    nc.sync.dma_start(
                        out=idx_t[:],
                        in_=idx_d[:, tok_base // 16 : (tok_base + 2 * btok) // 16],
                    )
                    msgs = []
                    for h in (0, 1):
                        mt = msg_pool.tile([P, bw * CH, table_elem], f16, tag=msg_tag)
                        lo = h * HALF
                        for off in range(0, btok, GN):
                            gn = min(GN, btok - off)
                            i0 = h * btok + off
                            nc.gpsimd.dma_gather(
                                out_ap=mt[:, off // P : (off + gn) // P, :],
                                in_ap=table[lo : lo + HALF, :],
                                idxs_ap=idx_t[:, i0 // 16 : (i0 + gn) // 16],
                                num_idxs=gn,
                                num_idxs_reg=gn,
                                elem_size=table_elem,
                                queue_num=qctr[0] % NQ,
                            )
                            qctr[0] += 1
                        msgs.append(mt)
                    for i, w in enumerate(bwins):
                        consume_window(w, i, bw, msgs, col_base)
                    tok_base += 2 * btok
                    col_base += 2 * bw * CH

            def build_G(col):
                G = gp.tile([P, P], f16, tag="G")
                nc.vector.tensor_tensor(
                    out=G[:],
                    in0=iota_t[:],
                    in1=grelS[:, col : col + 1].to_broadcast([P, P]),
                    op=mybir.AluOpType.is_equal,
                )
                return G

            def stage3_window(w, i, bw, msgs, col_base):
                pz = pp.tile([P, HID], f32, tag="acc256")
                nc.tensor.matmul(
                    pz[:], lhsT=invd[:1, ts(w, P)], rhs=b1s[:1, :],
                    start=True, stop=False,
                )
                for h in (0, 1):
                    for c in range(CH):
                        col = col_base + (h * bw + i) * CH + c
                        G = build_G(col)
                        nc.tensor.matmul(
                            pz[:],
                            lhsT=G[:],
                            rhs=msgs[h][:, i * CH + c, :],
                            start=False,
                            stop=(h == 1 and c == CH - 1),
                        )
                z1r = wp.tile([P, HID], f16, tag="z1r")
                nc.scalar.activation(
                    z1r[:], pz[:], mybir.ActivationFunctionType.Relu,
                    scale=disS[:, w : w + 1],
                )
                ph2 = pp.tile([P, OUTP], f32, tag="acc128b")
                for k in range(HC):
                    pt = pp.tile([P, P], f16, tag="acc128t")
                    nc.tensor.transpose(pt[:], z1r[:, ts(k, P)], ident[:])
                    zt = wp.tile([P, P], f16, tag="zt")
                    nc.vector.tensor_copy(zt[:], pt[:])
                    nc.tensor.matmul(
                        ph2[:],
                        lhsT=zt[:],
                        rhs=w2t[:, k, :],
                        start=(k == 0),
                        stop=(k == HC - 1),
                    )
                h2s = wp.tile([P, OUTP], f16, tag="h2s")
                nc.scalar.activation(
                    h2s[:], ph2[:], mybir.ActivationFunctionType.Copy,
                    scale=disS[:, w : w + 1],
                )
                nc.sync.dma_start(out=h2_loc[ts(w, P), :], in_=h2s[:])

            if "3" in stages:
                with tc.tile_pool(name="msg1", bufs=4) as mp1:
                    agg_batches(h1_gl, HID, "m1", mp1, stage3_window)

            # ---- stage 4: AllGather h2 ----
            if "4" in stages:
                nc.gpsimd.collective_compute(
                    "AllGather",
                    mybir.AluOpType.bypass,
                    replica_groups=rg,
                    ins=[h2_loc.ap().opt()],
                    outs=[h2_gl.ap().opt()],
                )

            # ---- stage 5: layer-2 aggregation ----
            def stage5_window(w, i, bw, msgs, col_base):
                po = pp.tile([P, OUTP], f32, tag="acc128b")
                nc.tensor.matmul(
                    po[:], lhsT=invd[:1, ts(w, P)], rhs=b2s[:1, :],
                    start=True, stop=False,
                )
                for h in (0, 1):
                    for c in range(CH):
                        col = col_base + (h * bw + i) * CH + c
                        G = build_G(col)
                        nc.tensor.matmul(
                            po[:],
                            lhsT=G[:],
                            rhs=msgs[h][:, i * CH + c, :],
                            start=False,
                            stop=(h == 1 and c == CH - 1),
                        )
                os_ = wp.tile([P, OUT], f32, tag="os")
                nc.scalar.activation(
                    os_[:], po[:, :OUT], mybir.ActivationFunctionType.Copy,
                    scale=disS[:, w : w + 1],
                )
                nc.sync.dma_start(out=out_d[ts(w, P), :], in_=os_[:])

            if "5" in stages:
                with tc.tile_pool(name="msg2", bufs=4) as mp2:
                    agg_batches(h2_gl, OUTP, "m2", mp2, stage5_window)
            else:
                zf = wp.tile([P, OUT], f32, tag="os")
                nc.gpsimd.memset(zf[:], 0.0)
                nc.sync.dma_start(out=out_d[0:P, :], in_=zf[:])

    nc.compile()
    return nc


def kernel(x, W1, b1, W2, b2, edge_index, _run_opts=None):
    from concourse.bass_utils import run_bass_kernel_spmd

    x = np.asarray(x)
    edge_index = np.asarray(edge_index)
    in_maps, meta = _prep(
        x, np.asarray(W1), np.asarray(b1), np.asarray(W2), np.asarray(b2), edge_index
    )
    nc = _build(meta)
    opts = _run_opts or {}
    res = run_bass_kernel_spmd(nc, in_maps, core_ids=list(range(M)), **opts)
    N, NP, OUT = meta["N"], meta["NP"], meta["OUT"]
    out = np.concatenate(
        [res.results[c]["out"][:NP] for c in range(M)], axis=0
    ).astype(np.float32)
    if _run_opts is not None:
        _run_opts["_bass_results"] = res
    return out


# revision 10
# speedup vs baseline: 1.5064x; 1.4648x over previous
"""GCN (2-layer, PyG GCNConv semantics) on 8 Trainium2 NeuronCores.

Strategy (sharding_hint: shard nodes across cores, partition edges by dst):
  - Nodes sharded contiguously: core c owns dst rows [c*NP, (c+1)*NP).
  - Layer matmuls computed on the owning core (fp16 operands, fp32 PSUM).
  - Hidden tables (h1 = dis*x@W1, h2 = dis*relu(z1)@W2) are AllGathered in
    4 pieces (overlapped with compute) so every core can gather messages
    for its own edges locally.
  - The symmetric norm dis[s]*dis[d] is factored: table rows are
    pre-scaled by dis[v]; the window PSUM is scaled by dis[d] on the way
    out (ACT scale); the bias is injected as (b/dis[d]) via a K=1 matmul
    that also initializes the accumulation group.
  - Aggregation out[dst] += h'[src_e] is done per 128-dst window:
    dma_gather pulls h'[src] rows for the window's non-self edges into
    SBUF (128 edges per chunk), a 0/1 selection mask G[e, d] =
    (dst_rel[e]==d) is built with one DVE tensor_tensor, and TensorE
    accumulates G.T @ msg into the window's PSUM tile.  Self-loops are
    the own-shard diagonal: one identity matmul on a contiguous DMA of
    the own h' tile.
  - All cores run one identical program: every (window, src-half) edge
    group is padded to CH chunks of 128 tokens (pad tokens have
    dst_rel=255 so the mask kills them).
"""

import math

import numpy as np

M = 8  # cores
P = 128  # partitions
AGP = 4  # all-gather pieces
BW = 4  # windows per gather batch


def _prep(x, W1, b1, W2, b2, edge_index):
    """Host-side sharding/layout (index manipulation + dtype casts only)."""
    N, IN = x.shape
    HID = W1.shape[1]
    OUT = W2.shape[1]
    OUTP = P
    assert N % M == 0
    NP = N // M
    NPAD = math.ceil(NP / P) * P
    NT = NPAD // P
    VROWS = M * NPAD
    HALF = VROWS // 2
    assert HALF <= 32768, "gather idx must fit int16"

    src = np.asarray(edge_index[0], dtype=np.int64)
    dst = np.asarray(edge_index[1], dtype=np.int64)
    # in-degree including the self-loop
    deg = (np.bincount(dst, minlength=N) + 1).astype(np.float32)
    dis = 1.0 / np.sqrt(deg)

    # gather batches (windows per batch) and all-gather pieces (batches per
    # piece); identical for every core.
    batches = [list(range(s, min(s + BW, NT))) for s in range(0, NT, BW)]
    nb = len(batches)
    pieces = []  # list of (batch_lo, batch_hi)
    per = math.ceil(nb / AGP)
    for s in range(0, nb, per):
        pieces.append((s, min(s + per, nb)))
    piece_wins = [
        sum(len(batches[b]) for b in range(lo, hi)) for lo, hi in pieces
    ]
    piece_rows = [wn * P for wn in piece_wins]
    piece_win_start = np.cumsum([0] + piece_wins)  # window index per piece
    piece_base = np.cumsum([0] + [r * M for r in piece_rows])  # global rows

    # remapped global row for node v = (c, l): AllGather piece layout
    win_of_l = np.arange(NPAD) // P
    piece_of_win = np.zeros(NT, dtype=np.int64)
    for j in range(len(pieces)):
        piece_of_win[piece_win_start[j] : piece_win_start[j + 1]] = j

    def grow(c, l):
        j = piece_of_win[win_of_l[l]]
        off = l - piece_win_start[j] * P
        return piece_base[j] + c * piece_rows[j] + off

    sc, sl = src // NP, src % NP
    g = piece_base[piece_of_win[win_of_l[sl]]] \
        + sc * np.array(piece_rows)[piece_of_win[win_of_l[sl]]] \
        + (sl - piece_win_start[piece_of_win[win_of_l[sl]]] * P)
    half = g // HALF
    lidx = g - half * HALF

    owner = dst // NP
    ldst = dst - owner * NP
    win = ldst // P

    gid = (owner * NT + win) * 2 + half
    order = np.lexsort((lidx, gid))  # sort by group, then ascending row
    gid_s = gid[order]
    lidx_s = lidx[order]
    drel_s = (ldst[order] % P).astype(np.float16)

    ngroups = M * NT * 2
    counts = np.bincount(gid_s, minlength=ngroups)
    CH = max(1, int(math.ceil(counts.max() / P)))
    GTOK = CH * P
    TOK = NT * 2 * GTOK
    NCHUNK = NT * 2 * CH

    slot_base = np.zeros((NT, 2), dtype=np.int64)
    tb = 0
    for bwins in batches:
        bw = len(bwins)
        for h in (0, 1):
            for i, w in enumerate(bwins):
                slot_base[w, h] = tb + i * GTOK
            tb += bw * GTOK
    assert tb == TOK

    group_starts = np.zeros(ngroups + 1, dtype=np.int64)
    np.cumsum(counts, out=group_starts[1:])

    in_maps = []
    f16 = np.float16
    w1f = np.ascontiguousarray(W1, dtype=f16)
    b1f = np.asarray(b1, dtype=f16).reshape(1, HID)
    w2f = np.zeros((HID, OUTP), dtype=f16)
    w2f[:, :OUT] = W2.astype(f16)
    b2f = np.zeros((1, OUTP), dtype=f16)
    b2f[0, :OUT] = np.asarray(b2, dtype=f16)
    iota_np = np.ascontiguousarray(np.broadcast_to(np.arange(P, dtype=f16), (P, P)))

    for c in range(M):
        xt = np.zeros((IN, NPAD), dtype=f16)
        xt[:, :NP] = x[c * NP : (c + 1) * NP].T
        idx16 = np.zeros(TOK, dtype=np.int16)
        drel = np.full(TOK, 255.0, dtype=np.float16)
        for w in range(NT):
            for h in (0, 1):
                gi = (c * NT + w) * 2 + h
                s0, s1 = group_starts[gi], group_starts[gi + 1]
                k = s1 - s0
                base = slot_base[w, h]
                idx16[base : base + k] = lidx_s[s0:s1]
                drel[base : base + k] = drel_s[s0:s1]
        idx_w = np.tile(idx16.reshape(-1, 16).T, (8, 1))
        grel = np.ascontiguousarray(drel.reshape(-1, P).T)
        dloc = np.ones(NPAD, np.float32)
        dloc[:NP] = dis[c * NP : (c + 1) * NP]
        disS = np.ascontiguousarray(dloc.reshape(NT, P).T)
        invd = (1.0 / dloc).astype(f16).reshape(1, NPAD)
        in_maps.append(
            {
                "xt": xt,
                "w1": w1f,
                "b1": b1f,
                "w2": w2f,
                "b2": b2f,
                "iota": np.array(iota_np),
                "idx": np.ascontiguousarray(idx_w),
                "grel": grel,
                "disS": disS,
                "invd": invd,
            }
        )

    meta = dict(
        N=N, IN=IN, HID=HID, OUT=OUT, OUTP=OUTP, NP=NP, NPAD=NPAD, NT=NT,
        VROWS=VROWS, HALF=HALF, CH=CH, TOK=TOK, NCHUNK=NCHUNK,
        batches=batches, pieces=pieces, piece_rows=piece_rows,
        piece_win_start=[int(v) for v in piece_win_start],
        piece_base=[int(v) for v in piece_base],
    )
    return in_maps, meta


def _build(meta):
    import os

    import concourse.mybir as mybir
    import concourse.tile as tile
    from concourse import bacc
    from concourse.bass import ts
    from concourse.masks import make_identity

    stages = os.environ.get("GCN_STAGES", "12345")

    IN, HID, OUT, OUTP = meta["IN"], meta["HID"], meta["OUT"], meta["OUTP"]
    NPAD, NT, VROWS, HALF = meta["NPAD"], meta["NT"], meta["VROWS"], meta["HALF"]
    CH, TOK, NCHUNK = meta["CH"], meta["TOK"], meta["NCHUNK"]
    batches = meta["batches"]
    pieces = meta["pieces"]
    piece_rows = meta["piece_rows"]
    piece_win_start = meta["piece_win_start"]
    piece_base = meta["piece_base"]
    NPIECE = len(pieces)
    KT = IN // P
    HC = HID // P
    f16 = mybir.dt.float16
    f32 = mybir.dt.float32

    NQ = 4
    GN = 1024
    nc = bacc.Bacc(
        "TRN2",
        target_bir_lowering=False,
        debug=False,
        num_devices=M,
        num_swdge_queues=NQ,
    )

    xt_d = nc.dram_tensor("xt", [IN, NPAD], f16, kind="ExternalInput")
    w1_d = nc.dram_tensor("w1", [IN, HID], f16, kind="ExternalInput")
    b1_d = nc.dram_tensor("b1", [1, HID], f16, kind="ExternalInput")
    w2_d = nc.dram_tensor("w2", [HID, OUTP], f16, kind="ExternalInput")
    b2_d = nc.dram_tensor("b2", [1, OUTP], f16, kind="ExternalInput")
    iota_d = nc.dram_tensor("iota", [P, P], f16, kind="ExternalInput")
    idx_d = nc.dram_tensor("idx", [P, TOK // 16], mybir.dt.int16, kind="ExternalInput")
    grel_d = nc.dram_tensor("grel", [P, NCHUNK], f16, kind="ExternalInput")
    disS_d = nc.dram_tensor("disS", [P, NT], f32, kind="ExternalInput")
    invd_d = nc.dram_tensor("invd", [1, NPAD], f16, kind="ExternalInput")
    out_d = nc.dram_tensor("out", [NPAD, OUT], f32, kind="ExternalOutput")

    # per-piece local tables (separate tensors => fine-grained deps for the
    # chunked all-gathers)
    h1_loc = [
        nc.dram_tensor(f"h1_loc{j}", [piece_rows[j], HID], f16)
        for j in range(NPIECE)
    ]
    h2_loc = [
        nc.dram_tensor(f"h2_loc{j}", [piece_rows[j], OUTP], f16)
        for j in range(NPIECE)
    ]
    h1_gl = nc.dram_tensor("h1_gl", [VROWS, HID], f16, addr_space="Shared")
    h2_gl = nc.dram_tensor("h2_gl", [VROWS, OUTP], f16, addr_space="Shared")

    rg = [list(range(M))]

    def win_piece(w):
        for j in range(NPIECE):
            if piece_win_start[j] <= w < piece_win_start[j + 1]:
                return j, w - piece_win_start[j]
        raise AssertionError(w)

    with tile.TileContext(nc) as tc:
        with (
            tc.tile_pool(name="const", bufs=1) as cp,
            tc.tile_pool(name="work", bufs=3) as wp,
            tc.tile_pool(name="gpool", bufs=6) as gp,
            tc.tile_pool(name="idxp", bufs=2) as idxp,
            tc.tile_pool(name="psum", bufs=2, space="PSUM") as pp,
        ):
            # ---- constants ----
            w1t = cp.tile([P, KT, HID], f16)
            nc.sync.dma_start(
                out=w1t[:], in_=w1_d[:, :].rearrange("(k p) h -> p k h", p=P)
            )
            w2t = cp.tile([P, HC, OUTP], f16)
            nc.sync.dma_start(
                out=w2t[:], in_=w2_d[:, :].rearrange("(k p) o -> p k o", p=P)
            )
            iota_t = cp.tile([P, P], f16)
            nc.sync.dma_start(out=iota_t[:], in_=iota_d[:, :])
            ident = cp.tile([P, P], f16)
            make_identity(nc, ident[:])
            b1s = cp.tile([1, HID], f16)
            nc.sync.dma_start(out=b1s[:], in_=b1_d[:, :])
            b2s = cp.tile([1, OUTP], f16)
            nc.sync.dma_start(out=b2s[:], in_=b2_d[:, :])
            grelS = cp.tile([P, NCHUNK], f16)
            nc.sync.dma_start(out=grelS[:], in_=grel_d[:, :])
            disS = cp.tile([P, NT], f32)
            nc.sync.dma_start(out=disS[:], in_=disS_d[:, :])
            invd = cp.tile([1, NPAD], f16)
            nc.sync.dma_start(out=invd[:], in_=invd_d[:, :])

            # ---- stage 1: h1' = dis * (x @ W1), written per piece ----
            if "1" in stages:
                for nt in range(NT):
                    j, wo = win_piece(nt)
                    xtt = wp.tile([P, KT, P], f16, tag="xtt")
                    nc.sync.dma_start(
                        out=xtt[:],
                        in_=xt_d[:, ts(nt, P)].rearrange("(k p) n -> p k n", p=P),
                    )
                    ph = pp.tile([P, HID], f32, tag="acc256")
                    for k in range(KT):
                        nc.tensor.matmul(
                            ph[:],
                            lhsT=xtt[:, k, :],
                            rhs=w1t[:, k, :],
                            start=(k == 0),
                            stop=(k == KT - 1),
                        )
                    h1s = wp.tile([P, HID], f16, tag="h1s")
                    nc.scalar.activation(
                        h1s[:], ph[:], mybir.ActivationFunctionType.Copy,
                        scale=disS[:, nt : nt + 1],
                    )
                    nc.sync.dma_start(out=h1_loc[j][ts(wo, P), :], in_=h1s[:])

            # ---- stage 2: chunked AllGather h1 ----
            if "2" in stages:
                for j in range(NPIECE):
                    nc.gpsimd.collective_compute(
                        "AllGather",
                        mybir.AluOpType.bypass,
                        replica_groups=rg,
                        ins=[h1_loc[j].ap().opt()],
                        outs=[
                            h1_gl[piece_base[j] : piece_base[j + 1], :].opt()
                        ],
                    )

            qctr = [0]

            def agg_batches(table, table_elem, msg_tag, msg_pool, consume_window):
                tok_base = 0
                col_base = 0
                for bwins in batches:
                    bw = len(bwins)
                    btok = bw * CH * P
                    idx_t = idxp.tile([P, 2 * btok // 16], mybir.dt.int16, tag="idx")
                    nc.sync.dma_start(
                        out=idx_t[:],
                        in_=idx_d[:, tok_base // 16 : (tok_base + 2 * btok) // 16],
                    )
                    msgs = []
                    for h in (0, 1):
                        mt = msg_pool.tile([P, bw * CH, table_elem], f16, tag=msg_tag)
                        lo = h * HALF
                        for off in range(0, btok, GN):
                            gn = min(GN, btok - off)
                            i0 = h * btok + off
                            nc.gpsimd.dma_gather(
                                out_ap=mt[:, off // P : (off + gn) // P, :],
                                in_ap=table[lo : lo + HALF, :],
                                idxs_ap=idx_t[:, i0 // 16 : (i0 + gn) // 16],
                                num_idxs=gn,
                                num_idxs_reg=gn,
                                elem_size=table_elem,
                                queue_num=qctr[0] % NQ,
                            )
                            qctr[0] += 1
                        msgs.append(mt)
                    for i, w in enumerate(bwins):
                        consume_window(w, i, bw, msgs, col_base)
                    tok_base += 2 * btok
                    col_base += 2 * bw * CH

            def build_G(col):
                G = gp.tile([P, P], f16, tag="G")
                nc.vector.tensor_tensor(
                    out=G[:],
                    in0=iota_t[:],
                    in1=grelS[:, col : col + 1].to_broadcast([P, P]),
                    op=mybir.AluOpType.is_equal,
                )
                return G

            def window_accum(w, i, bw, msgs, col_base, acc, bvec, own_loc, own_elem):
                """bias init + self-loop identity + edge chunks into PSUM."""
                j, wo = win_piece(w)
                nc.tensor.matmul(
                    acc[:], lhsT=invd[:1, ts(w, P)], rhs=bvec[:1, :],
                    start=True, stop=False,
                )
                own = wp.tile([P, own_elem], f16, tag=f"own{own_elem}")
                nc.sync.dma_start(out=own[:], in_=own_loc[j][ts(wo, P), :])
                nc.tensor.matmul(
                    acc[:], lhsT=ident[:], rhs=own[:], start=False, stop=False
                )
                for h in (0, 1):
                    for c in range(CH):
                        col = col_base + (h * bw + i) * CH + c
                        G = build_G(col)
                        nc.tensor.matmul(
                            acc[:],
                            lhsT=G[:],
                            rhs=msgs[h][:, i * CH + c, :],
                            start=False,
                            stop=(h == 1 and c == CH - 1),
                        )

            # ---- stage 3: layer-1 aggregation + fused layer-2 dense ----
            def stage3_window(w, i, bw, msgs, col_base):
                j, wo = win_piece(w)
                pz = pp.tile([P, HID], f32, tag="acc256")
                window_accum(w, i, bw, msgs, col_base, pz, b1s, h1_loc, HID)
                z1r = wp.tile([P, HID], f16, tag="z1r")
                nc.scalar.activation(
                    z1r[:], pz[:], mybir.ActivationFunctionType.Relu,
                    scale=disS[:, w : w + 1],
                )
                ph2 = pp.tile([P, OUTP], f32, tag="acc128b")
                for k in range(HC):
                    pt = pp.tile([P, P], f16, tag="acc128t")
                    nc.tensor.transpose(pt[:], z1r[:, ts(k, P)], ident[:])
                    zt = wp.tile([P, P], f16, tag="zt")
                    nc.vector.tensor_copy(zt[:], pt[:])
                    nc.tensor.matmul(
                        ph2[:],
                        lhsT=zt[:],
                        rhs=w2t[:, k, :],
                        start=(k == 0),
                        stop=(k == HC - 1),
                    )
                h2s = wp.tile([P, OUTP], f16, tag="h2s")
                # note: h2' = dis * relu(z1) @ W2  (no bias here; b2 is added
                # post-aggregation in stage 5)
                nc.scalar.activation(
                    h2s[:], ph2[:], mybir.ActivationFunctionType.Copy,
                    scale=disS[:, w : w + 1],
                )
                nc.sync.dma_start(out=h2_loc[j][ts(wo, P), :], in_=h2s[:])

            if "3" in stages:
                with tc.tile_pool(name="msg1", bufs=4) as mp1:
                    tok_base = 0
                    col_base = 0
                    done_pieces = set()
                    for bwins in batches:
                        bw = len(bwins)
                        btok = bw * CH * P
                        idx_t = idxp.tile(
                            [P, 2 * btok // 16], mybir.dt.int16, tag="idx"
                        )
                        nc.sync.dma_start(
                            out=idx_t[:],
                            in_=idx_d[
                                :, tok_base // 16 : (tok_base + 2 * btok) // 16
                            ],
                        )
                        msgs = []
                        for h in (0, 1):
                            mt = mp1.tile([P, bw * CH, HID], f16, tag="m1")
                            lo = h * HALF
                            for off in range(0, btok, GN):
                                gn = min(GN, btok - off)
                                i0 = h * btok + off
                                nc.gpsimd.dma_gather(
                                    out_ap=mt[:, off // P : (off + gn) // P, :],
                                    in_ap=h1_gl[lo : lo + HALF, :],
                                    idxs_ap=idx_t[:, i0 // 16 : (i0 + gn) // 16],
                                    num_idxs=gn,
                                    num_idxs_reg=gn,
                                    elem_size=HID,
                                    queue_num=qctr[0] % NQ,
                                )
                                qctr[0] += 1
                            msgs.append(mt)
                        for i, w in enumerate(bwins):
                            stage3_window(w, i, bw, msgs, col_base)
                            # chunked AllGather of h2 as pieces complete
                            j, wo = win_piece(w)
                            if (
                                "4" in stages
                                and wo == piece_rows[j] // P - 1
                                and j not in done_pieces
                            ):
                                done_pieces.add(j)
                                nc.gpsimd.collective_compute(
                                    "AllGather",
                                    mybir.AluOpType.bypass,
                                    replica_groups=rg,
                                    ins=[h2_loc[j].ap().opt()],
                                    outs=[
                                        h2_gl[
                                            piece_base[j] : piece_base[j + 1], :
                                        ].opt()
                                    ],
                                )
                        tok_base += 2 * btok
                        col_base += 2 * bw * CH

            # ---- stage 5: layer-2 aggregation ----
            def stage5_window(w, i, bw, msgs, col_base):
                po = pp.tile([P, OUTP], f32, tag="acc128b")
                window_accum(w, i, bw, msgs, col_base, po, b2s, h2_loc, OUTP)
                os_ = wp.tile([P, OUT], f32, tag="os")
                nc.scalar.activation(
                    os_[:], po[:, :OUT], mybir.ActivationFunctionType.Copy,
                    scale=disS[:, w : w + 1],
                )
                nc.sync.dma_start(out=out_d[ts(w, P), :], in_=os_[:])

            if "5" in stages:
                with tc.tile_pool(name="msg2", bufs=4) as mp2:
                    agg_batches(h2_gl, OUTP, "m2", mp2, stage5_window)
            else:
                zf = wp.tile([P, OUT], f32, tag="os")
                nc.gpsimd.memset(zf[:], 0.0)
                nc.sync.dma_start(out=out_d[0:P, :], in_=zf[:])

    nc.compile()
    return nc


def kernel(x, W1, b1, W2, b2, edge_index, _run_opts=None):
    from concourse.bass_utils import run_bass_kernel_spmd

    x = np.asarray(x)
    edge_index = np.asarray(edge_index)
    in_maps, meta = _prep(
        x, np.asarray(W1), np.asarray(b1), np.asarray(W2), np.asarray(b2), edge_index
    )
    nc = _build(meta)
    opts = dict(_run_opts or {})
    opts.pop("_bass_results", None)
    res = run_bass_kernel_spmd(nc, in_maps, core_ids=list(range(M)), **opts)
    NP, OUT = meta["NP"], meta["OUT"]
    out = np.concatenate(
        [res.results[c]["out"][:NP] for c in range(M)], axis=0
    ).astype(np.float32)
    if _run_opts is not None:
        _run_opts["_bass_results"] = res
    return out


# revision 11
# speedup vs baseline: 1.5936x; 1.0579x over previous
"""GCN (2-layer, PyG GCNConv semantics) on 8 Trainium2 NeuronCores.

Strategy (sharding_hint: shard nodes across cores, partition edges by dst):
  - Nodes sharded contiguously: core c owns dst rows [c*NP, (c+1)*NP).
  - Layer matmuls computed on the owning core (fp16 operands, fp32 PSUM).
  - Hidden tables (h1 = dis*x@W1, h2 = dis*relu(z1)@W2) are AllGathered in
    4 pieces (overlapped with compute) so every core can gather messages
    for its own edges locally.
  - The symmetric norm dis[s]*dis[d] is factored: table rows are
    pre-scaled by dis[v]; the window PSUM is scaled by dis[d] on the way
    out (ACT scale); the bias is injected as (b/dis[d]) via a K=1 matmul
    that also initializes the accumulation group.
  - Aggregation out[dst] += h'[src_e] is done per 128-dst window:
    dma_gather pulls h'[src] rows for the window's non-self edges into
    SBUF (128 edges per chunk), a 0/1 selection mask G[e, d] =
    (dst_rel[e]==d) is built with one DVE tensor_tensor, and TensorE
    accumulates G.T @ msg into the window's PSUM tile.  Self-loops are
    the own-shard diagonal: one identity matmul on a contiguous DMA of
    the own h' tile.
  - All cores run one identical program: every (window, src-half) edge
    group is padded to CH chunks of 128 tokens (pad tokens have
    dst_rel=255 so the mask kills them).
"""

import math

import numpy as np

M = 8  # cores
P = 128  # partitions
AGP = 4  # all-gather pieces
BW = 4  # windows per gather batch


def _prep(x, W1, b1, W2, b2, edge_index):
    """Host-side sharding/layout (index manipulation + dtype casts only)."""
    N, IN = x.shape
    HID = W1.shape[1]
    OUT = W2.shape[1]
    OUTP = P
    assert N % M == 0
    NP = N // M
    NPAD = math.ceil(NP / P) * P
    NT = NPAD // P
    VROWS = M * NPAD
    HALF = VROWS // 2
    assert HALF <= 32768, "gather idx must fit int16"

    src = np.asarray(edge_index[0], dtype=np.int64)
    dst = np.asarray(edge_index[1], dtype=np.int64)
    # in-degree including the self-loop
    deg = (np.bincount(dst, minlength=N) + 1).astype(np.float32)
    dis = 1.0 / np.sqrt(deg)

    # gather batches (windows per batch) and all-gather pieces (batches per
    # piece); identical for every core.
    batches = [list(range(s, min(s + BW, NT))) for s in range(0, NT, BW)]
    nb = len(batches)
    pieces = []  # list of (batch_lo, batch_hi)
    per = math.ceil(nb / AGP)
    for s in range(0, nb, per):
        pieces.append((s, min(s + per, nb)))
    piece_wins = [
        sum(len(batches[b]) for b in range(lo, hi)) for lo, hi in pieces
    ]
    piece_rows = [wn * P for wn in piece_wins]
    piece_win_start = np.cumsum([0] + piece_wins)  # window index per piece
    piece_base = np.cumsum([0] + [r * M for r in piece_rows])  # global rows

    # remapped global row for node v = (c, l): AllGather piece layout
    win_of_l = np.arange(NPAD) // P
    piece_of_win = np.zeros(NT, dtype=np.int64)
    for j in range(len(pieces)):
        piece_of_win[piece_win_start[j] : piece_win_start[j + 1]] = j

    def grow(c, l):
        j = piece_of_win[win_of_l[l]]
        off = l - piece_win_start[j] * P
        return piece_base[j] + c * piece_rows[j] + off

    sc, sl = src // NP, src % NP
    g = piece_base[piece_of_win[win_of_l[sl]]] \
        + sc * np.array(piece_rows)[piece_of_win[win_of_l[sl]]] \
        + (sl - piece_win_start[piece_of_win[win_of_l[sl]]] * P)
    half = g // HALF
    lidx = g - half * HALF

    owner = dst // NP
    ldst = dst - owner * NP
    win = ldst // P

    gid = (owner * NT + win) * 2 + half
    order = np.lexsort((lidx, gid))  # sort by group, then ascending row
    gid_s = gid[order]
    lidx_s = lidx[order]
    drel_s = (ldst[order] % P).astype(np.float16)

    ngroups = M * NT * 2
    counts = np.bincount(gid_s, minlength=ngroups)
    CH = max(1, int(math.ceil(counts.max() / P)))
    GTOK = CH * P
    TOK = NT * 2 * GTOK
    NCHUNK = NT * 2 * CH

    slot_base = np.zeros((NT, 2), dtype=np.int64)
    tb = 0
    for bwins in batches:
        bw = len(bwins)
        for h in (0, 1):
            for i, w in enumerate(bwins):
                slot_base[w, h] = tb + i * GTOK
            tb += bw * GTOK
    assert tb == TOK

    group_starts = np.zeros(ngroups + 1, dtype=np.int64)
    np.cumsum(counts, out=group_starts[1:])

    in_maps = []
    f16 = np.float16
    w1f = np.ascontiguousarray(W1, dtype=f16)
    b1f = np.asarray(b1, dtype=f16).reshape(1, HID)
    w2f = np.zeros((HID, OUTP), dtype=f16)
    w2f[:, :OUT] = W2.astype(f16)
    b2f = np.zeros((1, OUTP), dtype=f16)
    b2f[0, :OUT] = np.asarray(b2, dtype=f16)
    iota_np = np.ascontiguousarray(
        np.broadcast_to(
            np.tile(np.arange(P, dtype=f16), CH).reshape(1, CH * P), (P, CH * P)
        )
    )

    for c in range(M):
        xt = np.zeros((IN, NPAD), dtype=f16)
        xt[:, :NP] = x[c * NP : (c + 1) * NP].T
        idx16 = np.zeros(TOK, dtype=np.int16)
        drel = np.full(TOK, 255.0, dtype=np.float16)
        for w in range(NT):
            for h in (0, 1):
                gi = (c * NT + w) * 2 + h
                s0, s1 = group_starts[gi], group_starts[gi + 1]
                k = s1 - s0
                base = slot_base[w, h]
                idx16[base : base + k] = lidx_s[s0:s1]
                drel[base : base + k] = drel_s[s0:s1]
        idx_w = np.tile(idx16.reshape(-1, 16).T, (8, 1))
        grel = np.ascontiguousarray(drel.reshape(-1, P).T)
        dloc = np.ones(NPAD, np.float32)
        dloc[:NP] = dis[c * NP : (c + 1) * NP]
        disS = np.ascontiguousarray(dloc.reshape(NT, P).T)
        invd = (1.0 / dloc).astype(f16).reshape(1, NPAD)
        in_maps.append(
            {
                "xt": xt,
                "w1": w1f,
                "b1": b1f,
                "w2": w2f,
                "b2": b2f,
                "iota": np.array(iota_np),
                "idx": np.ascontiguousarray(idx_w),
                "grel": grel,
                "disS": disS,
                "invd": invd,
            }
        )

    meta = dict(
        N=N, IN=IN, HID=HID, OUT=OUT, OUTP=OUTP, NP=NP, NPAD=NPAD, NT=NT,
        VROWS=VROWS, HALF=HALF, CH=CH, TOK=TOK, NCHUNK=NCHUNK,
        batches=batches, pieces=pieces, piece_rows=piece_rows,
        piece_win_start=[int(v) for v in piece_win_start],
        piece_base=[int(v) for v in piece_base],
    )
    return in_maps, meta


def _build(meta):
    import os

    import concourse.mybir as mybir
    import concourse.tile as tile
    from concourse import bacc
    from concourse.bass import ts
    from concourse.masks import make_identity

    stages = os.environ.get("GCN_STAGES", "12345")

    IN, HID, OUT, OUTP = meta["IN"], meta["HID"], meta["OUT"], meta["OUTP"]
    NPAD, NT, VROWS, HALF = meta["NPAD"], meta["NT"], meta["VROWS"], meta["HALF"]
    CH, TOK, NCHUNK = meta["CH"], meta["TOK"], meta["NCHUNK"]
    batches = meta["batches"]
    pieces = meta["pieces"]
    piece_rows = meta["piece_rows"]
    piece_win_start = meta["piece_win_start"]
    piece_base = meta["piece_base"]
    NPIECE = len(pieces)
    KT = IN // P
    HC = HID // P
    f16 = mybir.dt.float16
    f32 = mybir.dt.float32

    NQ = 4
    GN = 1024
    SP = os.environ.get("GCN_SP", "1") == "1"
    nc = bacc.Bacc(
        "TRN2",
        target_bir_lowering=False,
        debug=False,
        num_devices=M,
        num_swdge_queues=NQ,
    )

    xt_d = nc.dram_tensor("xt", [IN, NPAD], f16, kind="ExternalInput")
    w1_d = nc.dram_tensor("w1", [IN, HID], f16, kind="ExternalInput")
    b1_d = nc.dram_tensor("b1", [1, HID], f16, kind="ExternalInput")
    w2_d = nc.dram_tensor("w2", [HID, OUTP], f16, kind="ExternalInput")
    b2_d = nc.dram_tensor("b2", [1, OUTP], f16, kind="ExternalInput")
    iota_d = nc.dram_tensor("iota", [P, CH * P], f16, kind="ExternalInput")
    idx_d = nc.dram_tensor("idx", [P, TOK // 16], mybir.dt.int16, kind="ExternalInput")
    grel_d = nc.dram_tensor("grel", [P, NCHUNK], f16, kind="ExternalInput")
    disS_d = nc.dram_tensor("disS", [P, NT], f32, kind="ExternalInput")
    invd_d = nc.dram_tensor("invd", [1, NPAD], f16, kind="ExternalInput")
    out_d = nc.dram_tensor("out", [NPAD, OUT], f32, kind="ExternalOutput")

    # per-piece local tables (separate tensors => fine-grained deps for the
    # chunked all-gathers)
    h1_loc = [
        nc.dram_tensor(f"h1_loc{j}", [piece_rows[j], HID], f16)
        for j in range(NPIECE)
    ]
    h2_loc = [
        nc.dram_tensor(f"h2_loc{j}", [piece_rows[j], OUTP], f16)
        for j in range(NPIECE)
    ]
    h1_gl = nc.dram_tensor("h1_gl", [VROWS, HID], f16, addr_space="Shared")
    h2_gl = nc.dram_tensor("h2_gl", [VROWS, OUTP], f16, addr_space="Shared")

    rg = [list(range(M))]

    def win_piece(w):
        for j in range(NPIECE):
            if piece_win_start[j] <= w < piece_win_start[j + 1]:
                return j, w - piece_win_start[j]
        raise AssertionError(w)

    with tile.TileContext(nc) as tc:
        with (
            tc.tile_pool(name="const", bufs=1) as cp,
            tc.tile_pool(name="work", bufs=3) as wp,
            tc.tile_pool(name="gpool", bufs=6) as gp,
            tc.tile_pool(name="idxp", bufs=2) as idxp,
            tc.tile_pool(name="psum", bufs=2, space="PSUM") as pp,
        ):
            # ---- constants ----
            w1t = cp.tile([P, KT, HID], f16)
            nc.sync.dma_start(
                out=w1t[:], in_=w1_d[:, :].rearrange("(k p) h -> p k h", p=P)
            )
            w2t = cp.tile([P, HC, OUTP], f16)
            nc.sync.dma_start(
                out=w2t[:], in_=w2_d[:, :].rearrange("(k p) o -> p k o", p=P)
            )
            iota_t = cp.tile([P, CH * P], f16)
            nc.sync.dma_start(out=iota_t[:], in_=iota_d[:, :])
            ident = cp.tile([P, P], f16)
            make_identity(nc, ident[:])
            b1s = cp.tile([1, HID], f16)
            nc.sync.dma_start(out=b1s[:], in_=b1_d[:, :])
            b2s = cp.tile([1, OUTP], f16)
            nc.sync.dma_start(out=b2s[:], in_=b2_d[:, :])
            grelS = cp.tile([P, NCHUNK], f16)
            nc.sync.dma_start(out=grelS[:], in_=grel_d[:, :])
            disS = cp.tile([P, NT], f32)
            nc.sync.dma_start(out=disS[:], in_=disS_d[:, :])
            invd = cp.tile([1, NPAD], f16)
            nc.sync.dma_start(out=invd[:], in_=invd_d[:, :])

            # ---- stage 1: h1' = dis * (x @ W1), written per piece ----
            if "1" in stages:
                for nt in range(NT):
                    j, wo = win_piece(nt)
                    xtt = wp.tile([P, KT, P], f16, tag="xtt")
                    nc.sync.dma_start(
                        out=xtt[:],
                        in_=xt_d[:, ts(nt, P)].rearrange("(k p) n -> p k n", p=P),
                    )
                    ph = pp.tile([P, HID], f32, tag="acc256")
                    for k in range(KT):
                        nc.tensor.matmul(
                            ph[:],
                            lhsT=xtt[:, k, :],
                            rhs=w1t[:, k, :],
                            start=(k == 0),
                            stop=(k == KT - 1),
                        )
                    h1s = wp.tile([P, HID], f16, tag="h1s")
                    nc.scalar.activation(
                        h1s[:], ph[:], mybir.ActivationFunctionType.Copy,
                        scale=disS[:, nt : nt + 1],
                    )
                    nc.sync.dma_start(out=h1_loc[j][ts(wo, P), :], in_=h1s[:])

            # ---- stage 2: chunked AllGather h1 ----
            if "2" in stages:
                for j in range(NPIECE):
                    nc.gpsimd.collective_compute(
                        "AllGather",
                        mybir.AluOpType.bypass,
                        replica_groups=rg,
                        ins=[h1_loc[j].ap().opt()],
                        outs=[
                            h1_gl[piece_base[j] : piece_base[j + 1], :].opt()
                        ],
                    )

            qctr = [0]

            def agg_batches(table, table_elem, msg_tag, msg_pool, consume_window):
                tok_base = 0
                col_base = 0
                for bwins in batches:
                    bw = len(bwins)
                    btok = bw * CH * P
                    idx_t = idxp.tile([P, 2 * btok // 16], mybir.dt.int16, tag="idx")
                    nc.sync.dma_start(
                        out=idx_t[:],
                        in_=idx_d[:, tok_base // 16 : (tok_base + 2 * btok) // 16],
                    )
                    msgs = []
                    for h in (0, 1):
                        mt = msg_pool.tile([P, bw * CH, table_elem], f16, tag=msg_tag)
                        lo = h * HALF
                        for off in range(0, btok, GN):
                            gn = min(GN, btok - off)
                            i0 = h * btok + off
                            nc.gpsimd.dma_gather(
                                out_ap=mt[:, off // P : (off + gn) // P, :],
                                in_ap=table[lo : lo + HALF, :],
                                idxs_ap=idx_t[:, i0 // 16 : (i0 + gn) // 16],
                                num_idxs=gn,
                                num_idxs_reg=gn,
                                elem_size=table_elem,
                                queue_num=qctr[0] % NQ,
                                single_packet=SP,
                            )
                            qctr[0] += 1
                        msgs.append(mt)
                    for i, w in enumerate(bwins):
                        consume_window(w, i, bw, msgs, col_base)
                    tok_base += 2 * btok
                    col_base += 2 * bw * CH

            def build_GW(c0):
                GW = gp.tile([P, CH * P], f16, tag="GW")
                nc.vector.tensor_tensor(
                    out=GW[:].rearrange("p (c e) -> p c e", e=P),
                    in0=iota_t[:].rearrange("p (c e) -> p c e", e=P),
                    in1=grelS[:, c0 : c0 + CH].to_broadcast([P, CH, P]),
                    op=mybir.AluOpType.is_equal,
                )
                return GW

            def window_accum(w, i, bw, msgs, col_base, acc, bvec, own_loc, own_elem):
                """bias init + self-loop identity + edge chunks into PSUM."""
                j, wo = win_piece(w)
                nc.tensor.matmul(
                    acc[:], lhsT=invd[:1, ts(w, P)], rhs=bvec[:1, :],
                    start=True, stop=False,
                )
                own = wp.tile([P, own_elem], f16, tag=f"own{own_elem}")
                nc.sync.dma_start(out=own[:], in_=own_loc[j][ts(wo, P), :])
                nc.tensor.matmul(
                    acc[:], lhsT=ident[:], rhs=own[:], start=False, stop=False
                )
                for h in (0, 1):
                    GW = build_GW(col_base + (h * bw + i) * CH)
                    for c in range(CH):
                        nc.tensor.matmul(
                            acc[:],
                            lhsT=GW[:, ts(c, P)],
                            rhs=msgs[h][:, i * CH + c, :],
                            start=False,
                            stop=(h == 1 and c == CH - 1),
                        )

            # ---- stage 3: layer-1 aggregation + fused layer-2 dense ----
            def stage3_window(w, i, bw, msgs, col_base):
                j, wo = win_piece(w)
                pz = pp.tile([P, HID], f32, tag="acc256")
                window_accum(w, i, bw, msgs, col_base, pz, b1s, h1_loc, HID)
                z1r = wp.tile([P, HID], f16, tag="z1r")
                nc.scalar.activation(
                    z1r[:], pz[:], mybir.ActivationFunctionType.Relu,
                    scale=disS[:, w : w + 1],
                )
                ph2 = pp.tile([P, OUTP], f32, tag="acc128b")
                for k in range(HC):
                    pt = pp.tile([P, P], f16, tag="acc128t")
                    nc.tensor.transpose(pt[:], z1r[:, ts(k, P)], ident[:])
                    zt = wp.tile([P, P], f16, tag="zt")
                    nc.vector.tensor_copy(zt[:], pt[:])
                    nc.tensor.matmul(
                        ph2[:],
                        lhsT=zt[:],
                        rhs=w2t[:, k, :],
                        start=(k == 0),
                        stop=(k == HC - 1),
                    )
                h2s = wp.tile([P, OUTP], f16, tag="h2s")
                # note: h2' = dis * relu(z1) @ W2  (no bias here; b2 is added
                # post-aggregation in stage 5)
                nc.scalar.activation(
                    h2s[:], ph2[:], mybir.ActivationFunctionType.Copy,
                    scale=disS[:, w : w + 1],
                )
                nc.sync.dma_start(out=h2_loc[j][ts(wo, P), :], in_=h2s[:])

            if "3" in stages:
                with tc.tile_pool(name="msg1", bufs=4) as mp1:
                    tok_base = 0
                    col_base = 0
                    done_pieces = set()
                    for bwins in batches:
                        bw = len(bwins)
                        btok = bw * CH * P
                        idx_t = idxp.tile(
                            [P, 2 * btok // 16], mybir.dt.int16, tag="idx"
                        )
                        nc.sync.dma_start(
                            out=idx_t[:],
                            in_=idx_d[
                                :, tok_base // 16 : (tok_base + 2 * btok) // 16
                            ],
                        )
                        msgs = []
                        for h in (0, 1):
                            mt = mp1.tile([P, bw * CH, HID], f16, tag="m1")
                            lo = h * HALF
                            for off in range(0, btok, GN):
                                gn = min(GN, btok - off)
                                i0 = h * btok + off
                                nc.gpsimd.dma_gather(
                                    out_ap=mt[:, off // P : (off + gn) // P, :],
                                    in_ap=h1_gl[lo : lo + HALF, :],
                                    idxs_ap=idx_t[:, i0 // 16 : (i0 + gn) // 16],
                                    num_idxs=gn,
                                    num_idxs_reg=gn,
                                    elem_size=HID,
                                    queue_num=qctr[0] % NQ,
                                    single_packet=SP,
                                )
                                qctr[0] += 1
                            msgs.append(mt)
                        for i, w in enumerate(bwins):
                            stage3_window(w, i, bw, msgs, col_base)
                            # chunked AllGather of h2 as pieces complete
                            j, wo = win_piece(w)
                            if (
                                "4" in stages
                                and wo == piece_rows[j] // P - 1
                                and j not in done_pieces
                            ):
                                done_pieces.add(j)
                                nc.gpsimd.collective_compute(
                                    "AllGather",
                                    mybir.AluOpType.bypass,
                                    replica_groups=rg,
                                    ins=[h2_loc[j].ap().opt()],
                                    outs=[
                                        h2_gl[
                                            piece_base[j] : piece_base[j + 1], :
                                        ].opt()
                                    ],
                                )
                        tok_base += 2 * btok
                        col_base += 2 * bw * CH

            # ---- stage 5: layer-2 aggregation ----
            def stage5_window(w, i, bw, msgs, col_base):
                po = pp.tile([P, OUTP], f32, tag="acc128b")
                window_accum(w, i, bw, msgs, col_base, po, b2s, h2_loc, OUTP)
                os_ = wp.tile([P, OUT], f32, tag="os")
                nc.scalar.activation(
                    os_[:], po[:, :OUT], mybir.ActivationFunctionType.Copy,
                    scale=disS[:, w : w + 1],
                )
                nc.sync.dma_start(out=out_d[ts(w, P), :], in_=os_[:])

            if "5" in stages:
                with tc.tile_pool(name="msg2", bufs=4) as mp2:
                    agg_batches(h2_gl, OUTP, "m2", mp2, stage5_window)
            else:
                zf = wp.tile([P, OUT], f32, tag="os")
                nc.gpsimd.memset(zf[:], 0.0)
                nc.sync.dma_start(out=out_d[0:P, :], in_=zf[:])

    nc.compile()
    return nc


def kernel(x, W1, b1, W2, b2, edge_index, _run_opts=None):
    from concourse.bass_utils import run_bass_kernel_spmd

    x = np.asarray(x)
    edge_index = np.asarray(edge_index)
    in_maps, meta = _prep(
        x, np.asarray(W1), np.asarray(b1), np.asarray(W2), np.asarray(b2), edge_index
    )
    nc = _build(meta)
    opts = dict(_run_opts or {})
    opts.pop("_bass_results", None)
    res = run_bass_kernel_spmd(nc, in_maps, core_ids=list(range(M)), **opts)
    NP, OUT = meta["NP"], meta["OUT"]
    out = np.concatenate(
        [res.results[c]["out"][:NP] for c in range(M)], axis=0
    ).astype(np.float32)
    if _run_opts is not None:
        _run_opts["_bass_results"] = res
    return out
